# revision 1
# baseline (speedup 1.0000x reference)
"""Trainium2 Bass kernel for LUT-based int8-quantized 3x3 conv (ApproxTorch baseline).

Problem: y = conv2d(quant(x), quant(w)) summed via a 256x256 LUT of int8
products, rescaled by (T_f/127)*(T_w/127) + bias, where T_f/T_w are EMA
thresholds updated with the *global* absmax of x / w before the conv.

Key observation: the LUT staged by setup_inputs() is the exact signed-product
table lut[a+128, b+128] = a*b, so the LUT-gather-sum is mathematically an
integer matmul; int8 values in [-128,127] are exactly representable in bf16 and
products accumulate exactly in fp32 PSUM (|sum| < 2^24), so the PE array
computes the reference bit-exactly. We verify the product-table property on the
host and refuse to run otherwise.

Sharding: data-parallel over batch (B=8 -> 1 image/core, 8 cores). Weights and
bias are replicated. The global absmax of x is computed via a tiny AllReduce
(max) across the 8 cores; absmax of w is local (w is replicated).

Per-core pipeline (one image, Cin=Cout=64, 28x28, 3x3, pad 1):
  1. DMA x [64,784] f32, w [64, 9*64] f32 (host pre-laid-out [Cin,(kh,kw),Cout]).
  2. DVE reduce abs-max -> [64,1] for x and w; PE-transpose -> rows; reduce ->
     local scalars; AllReduce(max) the x scalar across cores.
  3. Scalar row math: T = 0.95*T0 + 0.05*gmax; qscale = 127/T (bit-exact DVE
     reciprocal * 127); s = T*(1/127); ss = s_x*s_w. Broadcast (qs_x, qs_w, ss)
     to 64 partitions via a K=1 matmul with a ones row.
  4. Quantize (3 fused DVE tensor_scalar ops each): t=(v*qs+MAGIC);
     r=(t-MAGIC) max -128; q=r min 127 -> bf16. MAGIC=1.5*2^23 gives IEEE
     round-to-nearest-even, matching jnp.round. x is written into the interior
     of a zeroed [64,30,30] padded tile.
  5. 9 taps x 2 row-halves matmuls (K=Cin=64, M=Cout=64, N=392) accumulating in
     two PSUM banks.
  6. Epilogue: out = psum*ss + bias (one DVE tensor_scalar per half), DMA out.
"""

import os
import sys

import numpy as np

for _p in ("/opt/trn_rl_repo", "/root/.axon_site", "/root/.axon_site/_ro/trn_rl_repo",
           "/root/.axon_site/_ro/pypackages"):
    if os.path.isdir(_p) and _p not in sys.path:
        sys.path.append(_p)

from concourse import bacc, bass, masks, mybir, tile  # noqa: E402
from concourse.bass_utils import run_bass_kernel_spmd  # noqa: E402

F32 = mybir.dt.float32
BF16 = mybir.dt.bfloat16
AX = mybir.AxisListType
OP = mybir.AluOpType

N_CORES = 8
CIN = 64
COUT = 64
K = 3
H = W = 28
P = H * W          # 784 pixels
PH = P // 2        # 392, per PSUM-bank half (14 output rows)
R = CIN * K * K    # 576
PAD = 30           # padded spatial edge
MAGIC = 12582912.0  # 1.5 * 2**23: fp32 add/sub round-to-nearest-even trick

EMA_MUL = 0.05
T_CONSTS = (2.85, 0.285)  # 0.95*T_FEATURE, 0.95*T_WEIGHT as fp32
INV127 = float(np.float32(1.0) / np.float32(127.0))


def _build():
    nc = bacc.Bacc(
        "TRN2",
        target_bir_lowering=False,
        debug=False,
        enable_asserts=True,
        num_devices=N_CORES,
    )
    x_d = nc.dram_tensor("x", [CIN, P], F32, kind="ExternalInput")
    w_d = nc.dram_tensor("w", [CIN, K * K * COUT], F32, kind="ExternalInput")
    b_d = nc.dram_tensor("bias", [COUT], F32, kind="ExternalInput")
    c_d = nc.dram_tensor("consts", [2], F32, kind="ExternalInput")
    out_d = nc.dram_tensor("out", [COUT, P], F32, kind="ExternalOutput")

    with tile.TileContext(nc) as tc:
        with (
            tc.tile_pool(name="sbuf", bufs=1) as pool,
            tc.tile_pool(name="psum", bufs=1, space="PSUM") as psum,
            tc.tile_pool(name="dram", bufs=1, space="DRAM") as dram,
        ):
            x_sb = pool.tile([CIN, P], F32)
            w_sb = pool.tile([CIN, K * K * COUT], F32)
            bias_sb = pool.tile([COUT, 1], F32)
            crow = pool.tile([1, 2], F32)

            # ---- loads (split x across several DMAs for queue parallelism)
            for i in range(4):
                nc.sync.dma_start(out=x_sb[i * 16:(i + 1) * 16, :],
                                  in_=x_d[i * 16:(i + 1) * 16, :])
            for i in range(2):
                nc.sync.dma_start(out=w_sb[i * 32:(i + 1) * 32, :],
                                  in_=w_d[i * 32:(i + 1) * 32, :])
            nc.sync.dma_start(out=bias_sb[:],
                              in_=b_d.ap().rearrange("(p o) -> p o", o=1))
            nc.sync.dma_start(out=crow[:],
                              in_=c_d.ap().rearrange("(o t) -> o t", o=1))

            # ---- local absmax columns
            pack = pool.tile([CIN, 2], F32)
            nc.vector.tensor_reduce(out=pack[:, 0:1], in_=x_sb[:], axis=AX.X,
                                    op=OP.max, apply_absolute_value=True)
            nc.vector.tensor_reduce(out=pack[:, 1:2], in_=w_sb[:], axis=AX.X,
                                    op=OP.max, apply_absolute_value=True)

            ident = pool.tile([CIN, CIN], F32)
            masks.make_identity(nc, ident[:])

            rowT = psum.tile([2, CIN], F32)
            nc.tensor.transpose(rowT[:], pack[:], ident[:])
            colmax = pool.tile([2, 1], F32)
            nc.vector.tensor_reduce(out=colmax[:], in_=rowT[:], axis=AX.X,
                                    op=OP.max)
            lrow_ps = psum.tile([1, 2], F32)
            nc.tensor.transpose(lrow_ps[:], colmax[:], ident[0:2, 0:2])
            lrow = pool.tile([1, 2], F32)
            nc.vector.tensor_copy(lrow[:], lrow_ps[:])

            # ---- AllReduce(max) of (xmax, wmax) across the 8 cores
            cc_in = dram.tile([1, 2], F32)
            cc_out = dram.tile([1, 2], F32)
            nc.sync.dma_start(out=cc_in[:], in_=lrow[:])
            nc.gpsimd.collective_compute(
                "AllReduce",
                OP.max,
                replica_groups=[list(range(N_CORES))],
                ins=[cc_in[:]],
                outs=[cc_out[:]],
            )
            grow = pool.tile([1, 2], F32)
            nc.sync.dma_start(out=grow[:], in_=cc_out[:])

            # ---- scalar math in row form (partition 0)
            # T = grow*0.05 + (2.85, 0.285), two ops to force fp32 rounding
            t1 = pool.tile([1, 2], F32)
            nc.vector.tensor_scalar(out=t1[:], in0=grow[:], scalar1=EMA_MUL,
                                    scalar2=None, op0=OP.mult)
            trow = pool.tile([1, 2], F32)
            nc.vector.tensor_tensor(out=trow[:], in0=t1[:], in1=crow[:],
                                    op=OP.add)
            # qscale = 127 * (1/T); s = T * (1/127); ss = s_x * s_w
            recip = pool.tile([1, 2], F32)
            nc.vector.reciprocal(recip[:], trow[:])
            brow = pool.tile([1, 3], F32)
            nc.vector.tensor_scalar(out=brow[:, 0:2], in0=recip[:],
                                    scalar1=127.0, scalar2=None, op0=OP.mult)
            srow = pool.tile([1, 2], F32)
            nc.vector.tensor_scalar(out=srow[:], in0=trow[:], scalar1=INV127,
                                    scalar2=None, op0=OP.mult)
            nc.vector.tensor_tensor(out=brow[:, 2:3], in0=srow[:, 0:1],
                                    in1=srow[:, 1:2], op=OP.mult)

            # ---- broadcast (qs_x, qs_w, ss) to all 64 partitions via K=1 matmul
            ones = pool.tile([1, COUT], F32)
            nc.vector.memset(ones[:], 1.0)
            scl_ps = psum.tile([COUT, 3], F32)
            nc.tensor.matmul(scl_ps[:], ones[:], brow[:])
            scales = pool.tile([COUT, 3], F32)
            nc.vector.tensor_copy(scales[:], scl_ps[:])

            # ---- quantize x into zero-padded [64, 30, 30] bf16 tile
            qx_pad = pool.tile([CIN, PAD, PAD], BF16)
            nc.vector.memset(qx_pad[:], 0.0)
            tx = pool.tile([CIN, P], F32)
            nc.vector.tensor_scalar(out=tx[:], in0=x_sb[:],
                                    scalar1=scales[:, 0:1], scalar2=MAGIC,
                                    op0=OP.mult, op1=OP.add)
            rx = pool.tile([CIN, P], F32)
            nc.vector.tensor_scalar(out=rx[:], in0=tx[:], scalar1=MAGIC,
                                    scalar2=-128.0, op0=OP.subtract, op1=OP.max)
            nc.vector.tensor_scalar(
                out=qx_pad[:, 1:1 + H, 1:1 + W],
                in0=rx[:].rearrange("p (h w) -> p h w", h=H),
                scalar1=127.0, scalar2=None, op0=OP.min)

            # ---- quantize w -> bf16 [Cin, (kh,kw,Cout)]
            tw = pool.tile([CIN, K * K * COUT], F32)
            nc.vector.tensor_scalar(out=tw[:], in0=w_sb[:],
                                    scalar1=scales[:, 1:2], scalar2=MAGIC,
                                    op0=OP.mult, op1=OP.add)
            rw = pool.tile([CIN, K * K * COUT], F32)
            nc.vector.tensor_scalar(out=rw[:], in0=tw[:], scalar1=MAGIC,
                                    scalar2=-128.0, op0=OP.subtract, op1=OP.max)
            qw = pool.tile([CIN, K * K * COUT], BF16)
            nc.vector.tensor_scalar(out=qw[:], in0=rw[:], scalar1=127.0,
                                    scalar2=None, op0=OP.min)

            # ---- 9-tap conv as accumulated matmuls, two 14-row halves
            ph0 = psum.tile([COUT, PH], F32)
            ph1 = psum.tile([COUT, PH], F32)
            for t in range(K * K):
                kh, kw = divmod(t, K)
                lhsT = qw[:, t * COUT:(t + 1) * COUT]
                nc.tensor.matmul(
                    ph0[:], lhsT, qx_pad[:, kh:kh + 14, kw:kw + W],
                    start=(t == 0), stop=(t == K * K - 1))
                nc.tensor.matmul(
                    ph1[:], lhsT, qx_pad[:, kh + 14:kh + 28, kw:kw + W],
                    start=(t == 0), stop=(t == K * K - 1))

            # ---- epilogue: out = psum*ss + bias
            out_sb = pool.tile([COUT, P], F32)
            nc.vector.tensor_scalar(out=out_sb[:, 0:PH], in0=ph0[:],
                                    scalar1=scales[:, 2:3], scalar2=bias_sb[:],
                                    op0=OP.mult, op1=OP.add)
            nc.vector.tensor_scalar(out=out_sb[:, PH:P], in0=ph1[:],
                                    scalar1=scales[:, 2:3], scalar2=bias_sb[:],
                                    op0=OP.mult, op1=OP.add)
            for i in range(4):
                nc.sync.dma_start(out=out_d[i * 16:(i + 1) * 16, :],
                                  in_=out_sb[i * 16:(i + 1) * 16, :])

    nc.compile()
    return nc


_NC = None


def _get_nc():
    global _NC
    if _NC is None:
        _NC = _build()
    return _NC


def _prep_in_maps(x, weight, bias):
    # weight [Cout, Cin, Kh, Kw] -> lhsT layout [Cin, (kh, kw, Cout)]
    w_lhsT = np.ascontiguousarray(
        np.transpose(weight, (1, 2, 3, 0)).reshape(CIN, K * K * COUT))
    consts = np.array(T_CONSTS, dtype=np.float32)
    bias = np.ascontiguousarray(bias, dtype=np.float32)
    in_maps = []
    for b in range(N_CORES):
        in_maps.append({
            "x": np.ascontiguousarray(x[b].reshape(CIN, P), dtype=np.float32),
            "w": w_lhsT,
            "bias": bias,
            "consts": consts,
        })
    return in_maps


def _check_lut(lut):
    idx = np.arange(-128, 128, dtype=np.float32)
    expect = np.outer(idx, idx)
    if not np.array_equal(np.asarray(lut, dtype=np.float32), expect):
        raise ValueError(
            "lut is not the exact int8 product table; this kernel's PE-matmul "
            "formulation only applies to the exact-product LUT.")


def kernel(x, weight, bias, lut):
    _check_lut(lut)
    nc = _get_nc()
    in_maps = _prep_in_maps(np.asarray(x), np.asarray(weight), np.asarray(bias))
    res = run_bass_kernel_spmd(nc, in_maps, core_ids=list(range(N_CORES)))
    out = np.empty((N_CORES, COUT, H, W), dtype=np.float32)
    for b in range(N_CORES):
        out[b] = res.results[b]["out"].reshape(COUT, H, W)
    return out


# revision 2
# speedup vs baseline: 2.1518x; 2.1518x over previous
"""Trainium2 Bass kernel for LUT-based int8-quantized 3x3 conv (ApproxTorch baseline).

Problem: y = conv2d(quant(x), quant(w)) summed via a 256x256 LUT of int8
products, rescaled by (T_f/127)*(T_w/127) + bias, where T_f/T_w are EMA
thresholds updated with the *global* absmax of x / w before the conv.

Key observation: the LUT staged by setup_inputs() is the exact signed-product
table lut[a+128, b+128] = a*b, so the LUT-gather-sum is mathematically an
integer matmul; int8 values in [-128,127] are exactly representable in bf16 and
products accumulate exactly in fp32 PSUM (|sum| < 2^24), so the PE array
computes the reference bit-exactly. We verify the product-table property on the
host and refuse to run otherwise.

Sharding: data-parallel over batch (B=8 -> 1 image/core, 8 cores). Weights and
bias are replicated. The global absmax of x is computed redundantly on every
core from a full replica of x ([128, 3136] layout): +1.6 MB of DMA (~4.5 us)
beats the ~20 us mesh-AllReduce latency floor for a tiny collective and leaves
the kernel with zero cross-core dependencies (immune to core start skew).

PE packing: the 9 conv taps are paired along kh so the contraction uses all
128 PE rows. x is DMAed twice into a [128, 784] tile; the quantized copy in
partitions 64:127 is written one padded row UP, so a single [128, 14, 28]
moving AP feeds tap (0,kw) from the top half and tap (1,kw) from the bottom
half of the same window. 3 pair-groups (K=128) + 3 kh=2 singles (K=64), each
split into two 392-column PSUM banks -> 12 matmuls, 4704 streamed columns
(vs 18 / 7056 unpaired).

Per-core pipeline:
  1. DMA xall [128,3136] (8 chunks, absmax partial-reduced as chunks land),
     x twice into [128,784], wpair [128,384], bias, consts.
  2. absmax -> PE-transpose -> row scalars; EMA thresholds; qscale = 127*(1/T)
     (bit-exact DVE reciprocal), s = T*(1/127), ss = s_x*s_w; broadcast via
     K=1 matmul with a ones row.
  3. Quantize (ACT Copy(v*qs+MAGIC) -> DVE (t-MAGIC) max -128 -> min 127 to
     bf16; MAGIC = 1.5*2^23 gives IEEE round-to-nearest-even = jnp.round).
  4. 6 matmul groups x 2 PSUM halves, accumulate.
  5. out = psum*ss + bias (DVE), DMA out.
"""

import os
import sys

import numpy as np

for _p in ("/opt/trn_rl_repo", "/root/.axon_site", "/root/.axon_site/_ro/trn_rl_repo",
           "/root/.axon_site/_ro/pypackages"):
    if os.path.isdir(_p) and _p not in sys.path:
        sys.path.append(_p)

from concourse import bacc, bass, masks, mybir, tile  # noqa: E402
from concourse.bass_utils import run_bass_kernel_spmd  # noqa: E402

F32 = mybir.dt.float32
BF16 = mybir.dt.bfloat16
AX = mybir.AxisListType
OP = mybir.AluOpType
ACTF = mybir.ActivationFunctionType

N_CORES = 8
CIN = 64
COUT = 64
K = 3
H = W = 28
P = H * W            # 784 pixels
PH = P // 2          # 392 per PSUM bank (14 output rows)
PAD = 30             # padded spatial edge
NCHUNK = 8           # xall DMA/reduce chunks
XALL_P = 128
XALL_F = (N_CORES * CIN * P) // XALL_P  # 3136
NG = 6               # matmul groups: 3 kh-pairs + 3 kh=2 singles
MAGIC = 12582912.0   # 1.5 * 2**23: fp32 add/sub round-to-nearest-even trick

EMA_MUL = 0.05
T_CONSTS = (2.85, 0.285)  # 0.95*T_FEATURE, 0.95*T_WEIGHT as fp32
INV127 = float(np.float32(1.0) / np.float32(127.0))


def _build():
    nc = bacc.Bacc(
        "TRN2",
        target_bir_lowering=False,
        debug=False,
        enable_asserts=True,
        num_devices=N_CORES,
    )
    xall_d = nc.dram_tensor("xall", [XALL_P, XALL_F], F32, kind="ExternalInput")
    x_d = nc.dram_tensor("x", [CIN, P], F32, kind="ExternalInput")
    w_d = nc.dram_tensor("w", [2 * CIN, NG * COUT], F32, kind="ExternalInput")
    b_d = nc.dram_tensor("bias", [COUT], F32, kind="ExternalInput")
    c_d = nc.dram_tensor("consts", [2], F32, kind="ExternalInput")
    out_d = nc.dram_tensor("out", [COUT, P], F32, kind="ExternalOutput")

    with tile.TileContext(nc) as tc:
        with (
            tc.tile_pool(name="sbuf", bufs=1) as pool,
            tc.tile_pool(name="psum", bufs=1, space="PSUM") as psum,
        ):
            # ---- loads
            xall = pool.tile([XALL_P, XALL_F], F32)
            cf = XALL_F // NCHUNK
            for i in range(NCHUNK):
                nc.sync.dma_start(out=xall[:, i * cf:(i + 1) * cf],
                                  in_=xall_d[:, i * cf:(i + 1) * cf])
            x_sb = pool.tile([2 * CIN, P], F32)
            nc.sync.dma_start(out=x_sb[0:CIN, :], in_=x_d[:])
            nc.sync.dma_start(out=x_sb[CIN:2 * CIN, :], in_=x_d[:])
            w_sb = pool.tile([2 * CIN, NG * COUT], F32)
            nc.sync.dma_start(out=w_sb[:], in_=w_d[:])
            bias_sb = pool.tile([COUT, 1], F32)
            nc.sync.dma_start(out=bias_sb[:],
                              in_=b_d.ap().rearrange("(p o) -> p o", o=1))
            crow = pool.tile([1, 2], F32)
            nc.sync.dma_start(out=crow[:],
                              in_=c_d.ap().rearrange("(o t) -> o t", o=1))

            ident = pool.tile([XALL_P, XALL_P], F32)
            masks.make_identity(nc, ident[:])

            # ---- absmax: chunked partials for xall (overlap with DMA), w
            parts = pool.tile([XALL_P, NCHUNK], F32)
            for i in range(NCHUNK):
                nc.vector.tensor_reduce(out=parts[:, i:i + 1],
                                        in_=xall[:, i * cf:(i + 1) * cf],
                                        axis=AX.X, op=OP.max,
                                        apply_absolute_value=True)
            pack = pool.tile([XALL_P, 2], F32)
            nc.vector.tensor_reduce(out=pack[:, 0:1], in_=parts[:], axis=AX.X,
                                    op=OP.max)
            nc.vector.tensor_reduce(out=pack[:, 1:2], in_=w_sb[:], axis=AX.X,
                                    op=OP.max, apply_absolute_value=True)

            rowT = psum.tile([2, XALL_P], F32)
            nc.tensor.transpose(rowT[:], pack[:], ident[:])
            colmax = pool.tile([2, 1], F32)
            nc.vector.tensor_reduce(out=colmax[:], in_=rowT[:], axis=AX.X,
                                    op=OP.max)
            lrow_ps = psum.tile([1, 2], F32)
            nc.tensor.transpose(lrow_ps[:], colmax[:], ident[0:2, 0:2])
            grow = pool.tile([1, 2], F32)
            nc.vector.tensor_copy(grow[:], lrow_ps[:])

            # ---- scalar math in row form (partition 0)
            # T = grow*0.05 + (2.85, 0.285); two ops to force fp32 rounding
            t1 = pool.tile([1, 2], F32)
            nc.vector.tensor_scalar(out=t1[:], in0=grow[:], scalar1=EMA_MUL,
                                    scalar2=None, op0=OP.mult)
            trow = pool.tile([1, 2], F32)
            nc.vector.tensor_tensor(out=trow[:], in0=t1[:], in1=crow[:],
                                    op=OP.add)
            # qscale = 127*(1/T); s = T*(1/127); ss = s_x*s_w
            recip = pool.tile([1, 2], F32)
            nc.vector.reciprocal(recip[:], trow[:])
            brow = pool.tile([1, 3], F32)
            nc.vector.tensor_scalar(out=brow[:, 0:2], in0=recip[:],
                                    scalar1=127.0, scalar2=None, op0=OP.mult)
            srow = pool.tile([1, 2], F32)
            nc.vector.tensor_scalar(out=srow[:], in0=trow[:], scalar1=INV127,
                                    scalar2=None, op0=OP.mult)
            nc.vector.tensor_tensor(out=brow[:, 2:3], in0=srow[:, 0:1],
                                    in1=srow[:, 1:2], op=OP.mult)

            # ---- broadcast (qs_x, qs_w, ss) to all 128 partitions, K=1 matmul
            ones = pool.tile([1, XALL_P], F32)
            nc.vector.memset(ones[:], 1.0)
            scl_ps = psum.tile([XALL_P, 3], F32)
            nc.tensor.matmul(scl_ps[:], ones[:], brow[:])
            scales = pool.tile([XALL_P, 3], F32)
            nc.vector.tensor_copy(scales[:], scl_ps[:])

            # ---- quantize x (both partition halves at once)
            qx2 = pool.tile([2 * CIN, PAD, PAD], BF16)
            nc.vector.memset(qx2[:], 0.0)
            tx = pool.tile([2 * CIN, P], F32)
            nc.scalar.activation(tx[:], x_sb[:], ACTF.Copy,
                                 bias=MAGIC, scale=scales[:, 0:1])
            rx = pool.tile([2 * CIN, P], F32)
            nc.vector.tensor_scalar(out=rx[:], in0=tx[:], scalar1=MAGIC,
                                    scalar2=-128.0, op0=OP.subtract, op1=OP.max)
            # top half: normal pad-1 placement; bottom half: one padded row up
            nc.vector.tensor_scalar(
                out=qx2[0:CIN, 1:1 + H, 1:1 + W],
                in0=rx[0:CIN, :].rearrange("p (h w) -> p h w", h=H),
                scalar1=127.0, scalar2=None, op0=OP.min)
            nc.gpsimd.tensor_scalar(
                out=qx2[CIN:2 * CIN, 0:H, 1:1 + W],
                in0=rx[CIN:2 * CIN, :].rearrange("p (h w) -> p h w", h=H),
                scalar1=127.0, scalar2=None, op0=OP.min)

            # ---- quantize w
            tw = pool.tile([2 * CIN, NG * COUT], F32)
            nc.scalar.activation(tw[:], w_sb[:], ACTF.Copy,
                                 bias=MAGIC, scale=scales[:, 1:2])
            rw = pool.tile([2 * CIN, NG * COUT], F32)
            nc.vector.tensor_scalar(out=rw[:], in0=tw[:], scalar1=MAGIC,
                                    scalar2=-128.0, op0=OP.subtract, op1=OP.max)
            qw = pool.tile([2 * CIN, NG * COUT], BF16)
            nc.vector.tensor_scalar(out=qw[:], in0=rw[:], scalar1=127.0,
                                    scalar2=None, op0=OP.min)

            # ---- conv: 3 kh-pair groups (K=128) + 3 kh=2 singles (K=64)
            ph0 = psum.tile([COUT, PH], F32)
            ph1 = psum.tile([COUT, PH], F32)
            for g in range(NG):
                if g < 3:  # taps (0,kw) + (1,kw), kw = g
                    kh, kw, kp = 0, g, 2 * CIN
                else:      # tap (2,kw), kw = g - 3
                    kh, kw, kp = 2, g - 3, CIN
                lhsT = qw[0:kp, g * COUT:(g + 1) * COUT]
                nc.tensor.matmul(
                    ph0[:], lhsT, qx2[0:kp, kh:kh + 14, kw:kw + W],
                    start=(g == 0), stop=(g == NG - 1))
                nc.tensor.matmul(
                    ph1[:], lhsT, qx2[0:kp, kh + 14:kh + 28, kw:kw + W],
                    start=(g == 0), stop=(g == NG - 1))

            # ---- epilogue: out = psum*ss + bias
            out_sb = pool.tile([COUT, P], F32)
            nc.vector.tensor_scalar(out=out_sb[:, 0:PH], in0=ph0[:],
                                    scalar1=scales[0:COUT, 2:3],
                                    scalar2=bias_sb[:],
                                    op0=OP.mult, op1=OP.add)
            nc.vector.tensor_scalar(out=out_sb[:, PH:P], in0=ph1[:],
                                    scalar1=scales[0:COUT, 2:3],
                                    scalar2=bias_sb[:],
                                    op0=OP.mult, op1=OP.add)
            for i in range(4):
                nc.sync.dma_start(out=out_d[i * 16:(i + 1) * 16, :],
                                  in_=out_sb[i * 16:(i + 1) * 16, :])

    nc.compile()
    return nc


_NC = None


def _get_nc():
    global _NC
    if _NC is None:
        _NC = _build()
    return _NC


def _prep_in_maps(x, weight, bias):
    x = np.ascontiguousarray(x, dtype=np.float32)
    xall = x.reshape(XALL_P, XALL_F)
    # wpair [2*Cin, 6*Cout]: groups 0-2 pair taps (0,kw)/(1,kw) across the
    # partition halves; groups 3-5 hold tap (2,kw) in the top half only.
    wpair = np.zeros((2 * CIN, NG * COUT), dtype=np.float32)
    wt = np.transpose(weight, (1, 2, 3, 0))  # [Cin, kh, kw, Cout]
    for g in range(3):
        wpair[0:CIN, g * COUT:(g + 1) * COUT] = wt[:, 0, g, :]
        wpair[CIN:2 * CIN, g * COUT:(g + 1) * COUT] = wt[:, 1, g, :]
        wpair[0:CIN, (3 + g) * COUT:(4 + g) * COUT] = wt[:, 2, g, :]
    consts = np.array(T_CONSTS, dtype=np.float32)
    bias = np.ascontiguousarray(bias, dtype=np.float32)
    in_maps = []
    for b in range(N_CORES):
        in_maps.append({
            "xall": xall,
            "x": np.ascontiguousarray(x[b].reshape(CIN, P)),
            "w": wpair,
            "bias": bias,
            "consts": consts,
        })
    return in_maps


def _check_lut(lut):
    idx = np.arange(-128, 128, dtype=np.float32)
    expect = np.outer(idx, idx)
    if not np.array_equal(np.asarray(lut, dtype=np.float32), expect):
        raise ValueError(
            "lut is not the exact int8 product table; this kernel's PE-matmul "
            "formulation only applies to the exact-product LUT.")


def kernel(x, weight, bias, lut):
    _check_lut(lut)
    nc = _get_nc()
    in_maps = _prep_in_maps(np.asarray(x), np.asarray(weight), np.asarray(bias))
    res = run_bass_kernel_spmd(nc, in_maps, core_ids=list(range(N_CORES)))
    out = np.empty((N_CORES, COUT, H, W), dtype=np.float32)
    for b in range(N_CORES):
        out[b] = res.results[b]["out"].reshape(COUT, H, W)
    return out


# revision 7
# speedup vs baseline: 2.4595x; 1.1430x over previous
"""Trainium2 Bass kernel for LUT-based int8-quantized 3x3 conv (ApproxTorch baseline).

Problem: y = conv2d(quant(x), quant(w)) summed via a 256x256 LUT of int8
products, rescaled by (T_f/127)*(T_w/127) + bias, where T_f/T_w are EMA
thresholds updated with the *global* absmax of x / w before the conv.

Key observation: the LUT staged by setup_inputs() is the exact signed-product
table lut[a+128, b+128] = a*b, so the LUT-gather-sum is mathematically an
integer matmul; int8 values in [-128,127] are exactly representable in bf16 and
products accumulate exactly in fp32 PSUM (|sum| < 2^24), so the PE array
computes the reference bit-exactly. We verify the product-table property on the
host and refuse to run otherwise.

Sharding: data-parallel over batch (B=8 -> 1 image/core, 8 cores). Weights and
bias are replicated. The global absmax of x is computed redundantly on every
core from a full replica of x ([128, 3136] layout): +1.6 MB of DMA (~4.5 us)
beats the ~20 us mesh-AllReduce latency floor for a tiny collective and leaves
the kernel with zero cross-core dependencies (immune to core start skew).

PE packing: the 9 conv taps are paired along kh so the contraction uses all
128 PE rows. x is DMAed twice into a [128, 784] tile; the quantized copy in
partitions 64:127 is written one padded row UP, so a single [128, 14, 28]
moving AP feeds tap (0,kw) from the top half and tap (1,kw) from the bottom
half of the same window. 3 pair-groups (K=128) + 3 kh=2 singles (K=64), each
split into two 392-column PSUM banks -> 12 matmuls, 4704 streamed columns
(vs 18 / 7056 unpaired).

Per-core pipeline:
  1. DMA xall [128,3136] (8 chunks, absmax partial-reduced as chunks land),
     x twice into [128,784], wpair [128,384], bias, consts.
  2. absmax -> PE-transpose -> row scalars; EMA thresholds; qscale = 127*(1/T)
     (bit-exact DVE reciprocal), s = T*(1/127), ss = s_x*s_w; broadcast via
     K=1 matmul with a ones row.
  3. Quantize (ACT Copy(v*qs+MAGIC) -> DVE (t-MAGIC) max -128 -> min 127 to
     bf16; MAGIC = 1.5*2^23 gives IEEE round-to-nearest-even = jnp.round).
  4. 6 matmul groups x 2 PSUM halves, accumulate.
  5. out = psum*ss + bias (DVE), DMA out.
"""

import os
import sys

import numpy as np

for _p in ("/opt/trn_rl_repo", "/root/.axon_site", "/root/.axon_site/_ro/trn_rl_repo",
           "/root/.axon_site/_ro/pypackages"):
    if os.path.isdir(_p) and _p not in sys.path:
        sys.path.append(_p)

from concourse import bacc, bass, masks, mybir, tile  # noqa: E402
from concourse.bass_utils import run_bass_kernel_spmd  # noqa: E402

F32 = mybir.dt.float32
BF16 = mybir.dt.bfloat16
AX = mybir.AxisListType
OP = mybir.AluOpType
ACTF = mybir.ActivationFunctionType

N_CORES = 8
CIN = 64
COUT = 64
K = 3
H = W = 28
P = H * W            # 784 pixels
PH = P // 2          # 392 per PSUM bank (14 output rows)
PAD = 30             # padded spatial edge
NCHUNK = 8           # xall DMA/reduce chunks
XALL_P = 128
XALL_F = (N_CORES * CIN * P) // XALL_P  # 3136
NG = 6               # matmul groups: 3 kh-pairs + 3 kh=2 singles
MAGIC = 12582912.0   # 1.5 * 2**23: fp32 add/sub round-to-nearest-even trick

EMA_MUL = 0.05
T_CONSTS = (2.85, 0.285)  # 0.95*T_FEATURE, 0.95*T_WEIGHT as fp32
INV127 = float(np.float32(1.0) / np.float32(127.0))


def _build():
    nc = bacc.Bacc(
        "TRN2",
        target_bir_lowering=False,
        debug=False,
        enable_asserts=True,
        num_devices=N_CORES,
    )
    xall_d = nc.dram_tensor("xall", [XALL_P, XALL_F], F32, kind="ExternalInput")
    x_d = nc.dram_tensor("x", [CIN, PAD * PAD + PAD], F32, kind="ExternalInput")
    w_d = nc.dram_tensor("w", [2 * CIN, NG * COUT], F32, kind="ExternalInput")
    b_d = nc.dram_tensor("bias", [COUT], F32, kind="ExternalInput")
    c_d = nc.dram_tensor("consts", [2], F32, kind="ExternalInput")
    out_d = nc.dram_tensor("out", [COUT, P], F32, kind="ExternalOutput")

    with tile.TileContext(nc) as tc:
        with (
            tc.tile_pool(name="sbuf", bufs=1) as pool,
            tc.tile_pool(name="psum", bufs=1, space="PSUM") as psum,
        ):
            # ---- loads. xall: 2 column-halves x 8 partition-groups so each
            # DMA has 16 descriptors of 6272 contiguous bytes, and the absmax
            # partial-reduce of column-half h can start once its 8 groups land.
            xall = pool.tile([XALL_P, XALL_F], F32)
            cf = XALL_F // 2
            for h in range(2):
                for g in range(8):
                    nc.sync.dma_start(
                        out=xall[g * 16:(g + 1) * 16, h * cf:(h + 1) * cf],
                        in_=xall_d[g * 16:(g + 1) * 16, h * cf:(h + 1) * cf])
            # x arrives host-padded [64, 930]: [30,30] image pad-1 layout plus
            # 30 trailing zeros. Bottom half reads it at +30 (one padded row
            # up) so kh-pair matmul windows hit tap kh+1 in partitions 64:128.
            x_sb = pool.tile([2 * CIN, PAD * PAD], F32)
            for g in range(2):
                s = slice(g * 32, (g + 1) * 32)
                nc.sync.dma_start(out=x_sb[0:CIN, :][s, :],
                                  in_=x_d[:, 0:PAD * PAD][s, :])
                nc.sync.dma_start(out=x_sb[CIN:2 * CIN, :][s, :],
                                  in_=x_d[:, PAD:PAD * PAD + PAD][s, :])
            w_sb = pool.tile([2 * CIN, NG * COUT], F32)
            nc.sync.dma_start(out=w_sb[:], in_=w_d[:])
            bias_sb = pool.tile([COUT, 1], F32)
            nc.sync.dma_start(out=bias_sb[:],
                              in_=b_d.ap().rearrange("(p o) -> p o", o=1))
            crow = pool.tile([1, 2], F32)
            nc.sync.dma_start(out=crow[:],
                              in_=c_d.ap().rearrange("(o t) -> o t", o=1))

            ident = pool.tile([XALL_P, XALL_P], F32)
            masks.make_identity(nc, ident[:])

            # ---- absmax: one partial per column-half (overlaps DMA), then w
            parts = pool.tile([XALL_P, 2], F32)
            for h in range(2):
                nc.vector.tensor_reduce(out=parts[:, h:h + 1],
                                        in_=xall[:, h * cf:(h + 1) * cf],
                                        axis=AX.X, op=OP.max,
                                        apply_absolute_value=True)
            pack = pool.tile([XALL_P, 2], F32)
            nc.vector.tensor_reduce(out=pack[:, 0:1], in_=parts[:], axis=AX.X,
                                    op=OP.max)
            nc.vector.tensor_reduce(out=pack[:, 1:2], in_=w_sb[:], axis=AX.X,
                                    op=OP.max, apply_absolute_value=True)

            rowT = psum.tile([2, XALL_P], F32)
            nc.tensor.transpose(rowT[:], pack[:], ident[:])
            colmax = pool.tile([2, 1], F32)
            nc.vector.tensor_reduce(out=colmax[:], in_=rowT[:], axis=AX.X,
                                    op=OP.max)
            lrow_ps = psum.tile([1, 2], F32)
            nc.tensor.transpose(lrow_ps[:], colmax[:], ident[0:2, 0:2])
            grow = pool.tile([1, 2], F32)
            nc.vector.tensor_copy(grow[:], lrow_ps[:])

            # ---- scalar math in row form (partition 0)
            # T = grow*0.05 + (2.85, 0.285); two ops to force fp32 rounding
            t1 = pool.tile([1, 2], F32)
            nc.vector.tensor_scalar(out=t1[:], in0=grow[:], scalar1=EMA_MUL,
                                    scalar2=None, op0=OP.mult)
            trow = pool.tile([1, 2], F32)
            nc.vector.tensor_tensor(out=trow[:], in0=t1[:], in1=crow[:],
                                    op=OP.add)
            # qscale = 127*(1/T); s = T*(1/127); ss = s_x*s_w
            recip = pool.tile([1, 2], F32)
            nc.vector.reciprocal(recip[:], trow[:])
            brow = pool.tile([1, 3], F32)
            nc.vector.tensor_scalar(out=brow[:, 0:2], in0=recip[:],
                                    scalar1=127.0, scalar2=None, op0=OP.mult)
            srow = pool.tile([1, 2], F32)
            nc.vector.tensor_scalar(out=srow[:], in0=trow[:], scalar1=INV127,
                                    scalar2=None, op0=OP.mult)
            nc.vector.tensor_tensor(out=brow[:, 2:3], in0=srow[:, 0:1],
                                    in1=srow[:, 1:2], op=OP.mult)

            # ---- broadcast (qs_x, qs_w, ss) to all 128 partitions, K=1 matmul
            ones = pool.tile([1, XALL_P], F32)
            nc.vector.memset(ones[:], 1.0)
            scl_ps = psum.tile([XALL_P, 3], F32)
            nc.tensor.matmul(scl_ps[:], ones[:], brow[:])
            scales = pool.tile([XALL_P, 3], F32)
            nc.vector.tensor_copy(scales[:], scl_ps[:])

            # ---- quantize x: fully contiguous ops (padding quantizes to 0)
            tx = pool.tile([2 * CIN, PAD * PAD], F32)
            nc.scalar.activation(tx[:], x_sb[:], ACTF.Copy,
                                 bias=MAGIC, scale=scales[:, 0:1])
            rx = pool.tile([2 * CIN, PAD * PAD], F32)
            nc.vector.tensor_scalar(out=rx[:], in0=tx[:], scalar1=MAGIC,
                                    scalar2=-128.0, op0=OP.subtract, op1=OP.max)
            qx2f = pool.tile([2 * CIN, PAD * PAD], BF16)
            nc.vector.tensor_scalar(out=qx2f[:], in0=rx[:], scalar1=127.0,
                                    scalar2=None, op0=OP.min)
            qx2 = qx2f[:].rearrange("p (h w) -> p h w", h=PAD)

            # ---- quantize w
            tw = pool.tile([2 * CIN, NG * COUT], F32)
            nc.scalar.activation(tw[:], w_sb[:], ACTF.Copy,
                                 bias=MAGIC, scale=scales[:, 1:2])
            rw = pool.tile([2 * CIN, NG * COUT], F32)
            nc.vector.tensor_scalar(out=rw[:], in0=tw[:], scalar1=MAGIC,
                                    scalar2=-128.0, op0=OP.subtract, op1=OP.max)
            qw = pool.tile([2 * CIN, NG * COUT], BF16)
            nc.vector.tensor_scalar(out=qw[:], in0=rw[:], scalar1=127.0,
                                    scalar2=None, op0=OP.min)

            # ---- conv: 3 kh-pair groups (K=128) + 3 kh=2 singles (K=64)
            ph0 = psum.tile([COUT, PH], F32)
            ph1 = psum.tile([COUT, PH], F32)
            for g in range(NG):
                if g < 3:  # taps (0,kw) + (1,kw), kw = g
                    kh, kw, kp = 0, g, 2 * CIN
                else:      # tap (2,kw), kw = g - 3
                    kh, kw, kp = 2, g - 3, CIN
                lhsT = qw[0:kp, g * COUT:(g + 1) * COUT]
                nc.tensor.matmul(
                    ph0[:], lhsT, qx2[0:kp, kh:kh + 14, kw:kw + W],
                    start=(g == 0), stop=(g == NG - 1))
                nc.tensor.matmul(
                    ph1[:], lhsT, qx2[0:kp, kh + 14:kh + 28, kw:kw + W],
                    start=(g == 0), stop=(g == NG - 1))

            # ---- epilogue: out = psum*ss + bias
            out_sb = pool.tile([COUT, P], F32)
            nc.vector.tensor_scalar(out=out_sb[:, 0:PH], in0=ph0[:],
                                    scalar1=scales[0:COUT, 2:3],
                                    scalar2=bias_sb[:],
                                    op0=OP.mult, op1=OP.add)
            nc.vector.tensor_scalar(out=out_sb[:, PH:P], in0=ph1[:],
                                    scalar1=scales[0:COUT, 2:3],
                                    scalar2=bias_sb[:],
                                    op0=OP.mult, op1=OP.add)
            for i in range(4):
                nc.sync.dma_start(out=out_d[i * 16:(i + 1) * 16, :],
                                  in_=out_sb[i * 16:(i + 1) * 16, :])

    nc.compile()
    return nc


_NC = None


def _get_nc():
    global _NC
    if _NC is None:
        _NC = _build()
    return _NC


def _prep_in_maps(x, weight, bias):
    x = np.ascontiguousarray(x, dtype=np.float32)
    xall = x.reshape(XALL_P, XALL_F)
    # host-padded per-image layout [64, 30*30 + 30]: pad-1 image + 30 zeros
    xpad = np.zeros((N_CORES, CIN, PAD * PAD + PAD), dtype=np.float32)
    xpad[:, :, :PAD * PAD].reshape(N_CORES, CIN, PAD, PAD)[
        :, :, 1:1 + H, 1:1 + W] = x.reshape(N_CORES, CIN, H, W)
    # wpair [2*Cin, 6*Cout]: groups 0-2 pair taps (0,kw)/(1,kw) across the
    # partition halves; groups 3-5 hold tap (2,kw) in the top half only.
    wpair = np.zeros((2 * CIN, NG * COUT), dtype=np.float32)
    wt = np.transpose(weight, (1, 2, 3, 0))  # [Cin, kh, kw, Cout]
    for g in range(3):
        wpair[0:CIN, g * COUT:(g + 1) * COUT] = wt[:, 0, g, :]
        wpair[CIN:2 * CIN, g * COUT:(g + 1) * COUT] = wt[:, 1, g, :]
        wpair[0:CIN, (3 + g) * COUT:(4 + g) * COUT] = wt[:, 2, g, :]
    consts = np.array(T_CONSTS, dtype=np.float32)
    bias = np.ascontiguousarray(bias, dtype=np.float32)
    in_maps = []
    for b in range(N_CORES):
        in_maps.append({
            "xall": xall,
            "x": xpad[b],
            "w": wpair,
            "bias": bias,
            "consts": consts,
        })
    return in_maps


def _check_lut(lut):
    idx = np.arange(-128, 128, dtype=np.float32)
    expect = np.outer(idx, idx)
    if not np.array_equal(np.asarray(lut, dtype=np.float32), expect):
        raise ValueError(
            "lut is not the exact int8 product table; this kernel's PE-matmul "
            "formulation only applies to the exact-product LUT.")


def kernel(x, weight, bias, lut):
    _check_lut(lut)
    nc = _get_nc()
    in_maps = _prep_in_maps(np.asarray(x), np.asarray(weight), np.asarray(bias))
    res = run_bass_kernel_spmd(nc, in_maps, core_ids=list(range(N_CORES)))
    out = np.empty((N_CORES, COUT, H, W), dtype=np.float32)
    for b in range(N_CORES):
        out[b] = res.results[b]["out"].reshape(COUT, H, W)
    return out


# revision 9
# speedup vs baseline: 2.4917x; 1.0131x over previous
"""Trainium2 Bass kernel for LUT-based int8-quantized 3x3 conv (ApproxTorch baseline).

Problem: y = conv2d(quant(x), quant(w)) summed via a 256x256 LUT of int8
products, rescaled by (T_f/127)*(T_w/127) + bias, where T_f/T_w are EMA
thresholds updated with the *global* absmax of x / w before the conv.

Key observation: the LUT staged by setup_inputs() is the exact signed-product
table lut[a+128, b+128] = a*b, so the LUT-gather-sum is mathematically an
integer matmul; int8 values in [-128,127] are exactly representable in bf16 and
products accumulate exactly in fp32 PSUM (|sum| < 2^24), so the PE array
computes the reference bit-exactly. We verify the product-table property on the
host and refuse to run otherwise.

Sharding: data-parallel over batch (B=8 -> 1 image/core, 8 cores). Weights and
bias are replicated. The global absmax of x is computed redundantly on every
core from a full replica of x ([128, 3136] layout): +1.6 MB of DMA (~4.5 us)
beats the ~20 us mesh-AllReduce latency floor for a tiny collective and leaves
the kernel with zero cross-core dependencies (immune to core start skew).

PE packing: the 9 conv taps are paired along kh so the contraction uses all
128 PE rows. x is DMAed twice into a [128, 784] tile; the quantized copy in
partitions 64:127 is written one padded row UP, so a single [128, 14, 28]
moving AP feeds tap (0,kw) from the top half and tap (1,kw) from the bottom
half of the same window. 3 pair-groups (K=128) + 3 kh=2 singles (K=64), each
split into two 392-column PSUM banks -> 12 matmuls, 4704 streamed columns
(vs 18 / 7056 unpaired).

Per-core pipeline:
  1. DMA xall [128,3136] (8 chunks, absmax partial-reduced as chunks land),
     x twice into [128,784], wpair [128,384], bias, consts.
  2. absmax -> PE-transpose -> row scalars; EMA thresholds; qscale = 127*(1/T)
     (bit-exact DVE reciprocal), s = T*(1/127), ss = s_x*s_w; broadcast via
     K=1 matmul with a ones row.
  3. Quantize (ACT Copy(v*qs+MAGIC) -> DVE (t-MAGIC) max -128 -> min 127 to
     bf16; MAGIC = 1.5*2^23 gives IEEE round-to-nearest-even = jnp.round).
  4. 6 matmul groups x 2 PSUM halves, accumulate.
  5. out = psum*ss + bias (DVE), DMA out.
"""

import os
import sys

import numpy as np

for _p in ("/opt/trn_rl_repo", "/root/.axon_site", "/root/.axon_site/_ro/trn_rl_repo",
           "/root/.axon_site/_ro/pypackages"):
    if os.path.isdir(_p) and _p not in sys.path:
        sys.path.append(_p)

from concourse import bacc, bass, masks, mybir, tile  # noqa: E402
from concourse.bass_utils import run_bass_kernel_spmd  # noqa: E402

F32 = mybir.dt.float32
BF16 = mybir.dt.bfloat16
AX = mybir.AxisListType
OP = mybir.AluOpType
ACTF = mybir.ActivationFunctionType

N_CORES = 8
CIN = 64
COUT = 64
K = 3
H = W = 28
P = H * W            # 784 pixels
PH = P // 2          # 392 per PSUM bank (14 output rows)
PAD = 30             # padded spatial edge
NCHUNK = 8           # xall DMA/reduce chunks
XALL_P = 128
XALL_F = (N_CORES * CIN * P) // XALL_P  # 3136
NG = 6               # matmul groups: 3 kh-pairs + 3 kh=2 singles
MAGIC = 12582912.0   # 1.5 * 2**23: fp32 add/sub round-to-nearest-even trick

EMA_MUL = 0.05
T_CONSTS = (2.85, 0.285)  # 0.95*T_FEATURE, 0.95*T_WEIGHT as fp32
INV127 = float(np.float32(1.0) / np.float32(127.0))


def _build():
    nc = bacc.Bacc(
        "TRN2",
        target_bir_lowering=False,
        debug=False,
        enable_asserts=True,
        num_devices=N_CORES,
    )
    xall_d = nc.dram_tensor("xall", [XALL_P, XALL_F], F32, kind="ExternalInput")
    x_d = nc.dram_tensor("x", [CIN, PAD * PAD + PAD], F32, kind="ExternalInput")
    w_d = nc.dram_tensor("w", [2 * CIN, NG * COUT], F32, kind="ExternalInput")
    b_d = nc.dram_tensor("bias", [COUT], F32, kind="ExternalInput")
    c_d = nc.dram_tensor("consts", [2], F32, kind="ExternalInput")
    out_d = nc.dram_tensor("out", [COUT, P], F32, kind="ExternalOutput")

    with tile.TileContext(nc) as tc:
        with (
            tc.tile_pool(name="sbuf", bufs=1) as pool,
            tc.tile_pool(name="psum", bufs=1, space="PSUM") as psum,
        ):
            # ---- loads. Each dma_start spans all 128 partitions so its
            # descriptors fan out across all 16 SDMA engines (8 partitions
            # per engine); 2 column-halves let the absmax partial-reduce of
            # half h start as soon as that half lands.
            w_sb = pool.tile([2 * CIN, NG * COUT], F32)
            nc.sync.dma_start(out=w_sb[:], in_=w_d[:])
            xall = pool.tile([XALL_P, XALL_F], F32)
            cf = XALL_F // 2
            for h in range(2):
                nc.sync.dma_start(out=xall[:, h * cf:(h + 1) * cf],
                                  in_=xall_d[:, h * cf:(h + 1) * cf])
            # x arrives host-padded [64, 930]: [30,30] image pad-1 layout plus
            # 30 trailing zeros. Bottom half reads it at +30 (one padded row
            # up) so kh-pair matmul windows hit tap kh+1 in partitions 64:128.
            x_sb = pool.tile([2 * CIN, PAD * PAD], F32)
            nc.sync.dma_start(out=x_sb[0:CIN, :], in_=x_d[:, 0:PAD * PAD])
            nc.sync.dma_start(out=x_sb[CIN:2 * CIN, :],
                              in_=x_d[:, PAD:PAD * PAD + PAD])
            bias_sb = pool.tile([COUT, 1], F32)
            nc.sync.dma_start(out=bias_sb[:],
                              in_=b_d.ap().rearrange("(p o) -> p o", o=1))
            crow = pool.tile([1, 2], F32)
            nc.sync.dma_start(out=crow[:],
                              in_=c_d.ap().rearrange("(o t) -> o t", o=1))

            ident = pool.tile([XALL_P, XALL_P], F32)
            masks.make_identity(nc, ident[:])

            # ---- absmax: w first (small DMA, runs during xall transfer),
            # then one partial per xall column-half (overlaps its DMA)
            pack = pool.tile([XALL_P, 2], F32)
            nc.vector.tensor_reduce(out=pack[:, 1:2], in_=w_sb[:], axis=AX.X,
                                    op=OP.max, apply_absolute_value=True)
            parts = pool.tile([XALL_P, 2], F32)
            for h in range(2):
                nc.vector.tensor_reduce(out=parts[:, h:h + 1],
                                        in_=xall[:, h * cf:(h + 1) * cf],
                                        axis=AX.X, op=OP.max,
                                        apply_absolute_value=True)
            nc.vector.tensor_reduce(out=pack[:, 0:1], in_=parts[:], axis=AX.X,
                                    op=OP.max)

            rowT = psum.tile([2, XALL_P], F32)
            nc.tensor.transpose(rowT[:], pack[:], ident[:])
            colmax = pool.tile([2, 1], F32)
            nc.vector.tensor_reduce(out=colmax[:], in_=rowT[:], axis=AX.X,
                                    op=OP.max)
            lrow_ps = psum.tile([1, 2], F32)
            nc.tensor.transpose(lrow_ps[:], colmax[:], ident[0:2, 0:2])
            grow = pool.tile([1, 2], F32)
            nc.vector.tensor_copy(grow[:], lrow_ps[:])

            # ---- scalar math in row form (partition 0)
            # T = grow*0.05 + (2.85, 0.285); two ops to force fp32 rounding
            t1 = pool.tile([1, 2], F32)
            nc.vector.tensor_scalar(out=t1[:], in0=grow[:], scalar1=EMA_MUL,
                                    scalar2=None, op0=OP.mult)
            trow = pool.tile([1, 2], F32)
            nc.vector.tensor_tensor(out=trow[:], in0=t1[:], in1=crow[:],
                                    op=OP.add)
            # qscale = 127*(1/T); s = T*(1/127); ss = s_x*s_w
            recip = pool.tile([1, 2], F32)
            nc.vector.reciprocal(recip[:], trow[:])
            brow = pool.tile([1, 3], F32)
            nc.vector.tensor_scalar(out=brow[:, 0:2], in0=recip[:],
                                    scalar1=127.0, scalar2=None, op0=OP.mult)
            srow = pool.tile([1, 2], F32)
            nc.vector.tensor_scalar(out=srow[:], in0=trow[:], scalar1=INV127,
                                    scalar2=None, op0=OP.mult)
            nc.vector.tensor_tensor(out=brow[:, 2:3], in0=srow[:, 0:1],
                                    in1=srow[:, 1:2], op=OP.mult)

            # ---- broadcast (qs_x, qs_w, ss) to all 128 partitions, K=1 matmul
            ones = pool.tile([1, XALL_P], F32)
            nc.vector.memset(ones[:], 1.0)
            scl_ps = psum.tile([XALL_P, 3], F32)
            nc.tensor.matmul(scl_ps[:], ones[:], brow[:])
            scales = pool.tile([XALL_P, 3], F32)
            nc.vector.tensor_copy(scales[:], scl_ps[:])

            # ---- quantize x: fully contiguous ops (padding quantizes to 0)
            tx = pool.tile([2 * CIN, PAD * PAD], F32)
            nc.scalar.activation(tx[:], x_sb[:], ACTF.Copy,
                                 bias=MAGIC, scale=scales[:, 0:1])
            rx = pool.tile([2 * CIN, PAD * PAD], F32)
            nc.vector.tensor_scalar(out=rx[:], in0=tx[:], scalar1=MAGIC,
                                    scalar2=-128.0, op0=OP.subtract, op1=OP.max)
            qx2f = pool.tile([2 * CIN, PAD * PAD], BF16)
            nc.vector.tensor_scalar(out=qx2f[:], in0=rx[:], scalar1=127.0,
                                    scalar2=None, op0=OP.min)
            qx2 = qx2f[:].rearrange("p (h w) -> p h w", h=PAD)

            # ---- quantize w (gpsimd for steps 1+3 keeps ACT/DVE on the x path)
            tw = pool.tile([2 * CIN, NG * COUT], F32)
            nc.gpsimd.tensor_scalar(out=tw[:], in0=w_sb[:],
                                    scalar1=scales[:, 1:2], scalar2=MAGIC,
                                    op0=OP.mult, op1=OP.add)
            rw = pool.tile([2 * CIN, NG * COUT], F32)
            nc.vector.tensor_scalar(out=rw[:], in0=tw[:], scalar1=MAGIC,
                                    scalar2=-128.0, op0=OP.subtract, op1=OP.max)
            qw = pool.tile([2 * CIN, NG * COUT], BF16)
            nc.gpsimd.tensor_scalar(out=qw[:], in0=rw[:], scalar1=127.0,
                                    scalar2=None, op0=OP.min)

            # ---- conv: 3 kh-pair groups (K=128) + 3 kh=2 singles (K=64)
            ph0 = psum.tile([COUT, PH], F32)
            ph1 = psum.tile([COUT, PH], F32)
            for g in range(NG):
                if g < 3:  # taps (0,kw) + (1,kw), kw = g
                    kh, kw, kp = 0, g, 2 * CIN
                else:      # tap (2,kw), kw = g - 3
                    kh, kw, kp = 2, g - 3, CIN
                lhsT = qw[0:kp, g * COUT:(g + 1) * COUT]
                nc.tensor.matmul(
                    ph0[:], lhsT, qx2[0:kp, kh:kh + 14, kw:kw + W],
                    start=(g == 0), stop=(g == NG - 1))
                nc.tensor.matmul(
                    ph1[:], lhsT, qx2[0:kp, kh + 14:kh + 28, kw:kw + W],
                    start=(g == 0), stop=(g == NG - 1))

            # ---- epilogue: out = psum*ss + bias
            out_sb = pool.tile([COUT, P], F32)
            nc.vector.tensor_scalar(out=out_sb[:, 0:PH], in0=ph0[:],
                                    scalar1=scales[0:COUT, 2:3],
                                    scalar2=bias_sb[:],
                                    op0=OP.mult, op1=OP.add)
            nc.vector.tensor_scalar(out=out_sb[:, PH:P], in0=ph1[:],
                                    scalar1=scales[0:COUT, 2:3],
                                    scalar2=bias_sb[:],
                                    op0=OP.mult, op1=OP.add)
            for i in range(4):
                nc.sync.dma_start(out=out_d[i * 16:(i + 1) * 16, :],
                                  in_=out_sb[i * 16:(i + 1) * 16, :])

    nc.compile()
    return nc


_NC = None


def _get_nc():
    global _NC
    if _NC is None:
        _NC = _build()
    return _NC


def _prep_in_maps(x, weight, bias):
    x = np.ascontiguousarray(x, dtype=np.float32)
    xall = x.reshape(XALL_P, XALL_F)
    # host-padded per-image layout [64, 30*30 + 30]: pad-1 image + 30 zeros
    xpad = np.zeros((N_CORES, CIN, PAD * PAD + PAD), dtype=np.float32)
    xpad[:, :, :PAD * PAD].reshape(N_CORES, CIN, PAD, PAD)[
        :, :, 1:1 + H, 1:1 + W] = x.reshape(N_CORES, CIN, H, W)
    # wpair [2*Cin, 6*Cout]: groups 0-2 pair taps (0,kw)/(1,kw) across the
    # partition halves; groups 3-5 hold tap (2,kw) in the top half only.
    wpair = np.zeros((2 * CIN, NG * COUT), dtype=np.float32)
    wt = np.transpose(weight, (1, 2, 3, 0))  # [Cin, kh, kw, Cout]
    for g in range(3):
        wpair[0:CIN, g * COUT:(g + 1) * COUT] = wt[:, 0, g, :]
        wpair[CIN:2 * CIN, g * COUT:(g + 1) * COUT] = wt[:, 1, g, :]
        wpair[0:CIN, (3 + g) * COUT:(4 + g) * COUT] = wt[:, 2, g, :]
    consts = np.array(T_CONSTS, dtype=np.float32)
    bias = np.ascontiguousarray(bias, dtype=np.float32)
    in_maps = []
    for b in range(N_CORES):
        in_maps.append({
            "xall": xall,
            "x": xpad[b],
            "w": wpair,
            "bias": bias,
            "consts": consts,
        })
    return in_maps


def _check_lut(lut):
    idx = np.arange(-128, 128, dtype=np.float32)
    expect = np.outer(idx, idx)
    if not np.array_equal(np.asarray(lut, dtype=np.float32), expect):
        raise ValueError(
            "lut is not the exact int8 product table; this kernel's PE-matmul "
            "formulation only applies to the exact-product LUT.")


def kernel(x, weight, bias, lut):
    _check_lut(lut)
    nc = _get_nc()
    in_maps = _prep_in_maps(np.asarray(x), np.asarray(weight), np.asarray(bias))
    res = run_bass_kernel_spmd(nc, in_maps, core_ids=list(range(N_CORES)))
    out = np.empty((N_CORES, COUT, H, W), dtype=np.float32)
    for b in range(N_CORES):
        out[b] = res.results[b]["out"].reshape(COUT, H, W)
    return out


# revision 17
# speedup vs baseline: 3.0313x; 1.2166x over previous
"""Trainium2 Bass kernel for LUT-based int8-quantized 3x3 conv (ApproxTorch baseline).

Problem: y = conv2d(quant(x), quant(w)) summed via a 256x256 LUT of int8
products, rescaled by (T_f/127)*(T_w/127) + bias, where T_f/T_w are EMA
thresholds updated with the *global* absmax of x / w before the conv.

Key observation: the LUT staged by setup_inputs() is the exact signed-product
table lut[a+128, b+128] = a*b, so the LUT-gather-sum is mathematically an
integer matmul; int8 values in [-128,127] are exactly representable in bf16 and
products accumulate exactly in fp32 PSUM (|sum| < 2^24), so the PE array
computes the reference bit-exactly. We verify the product-table property on the
host and refuse to run otherwise.

Sharding: data-parallel over batch (B=8 -> 1 image/core, 8 cores). Weights and
bias are replicated. The global absmax of x is computed redundantly on every
core from a full replica of x ([128, 3136] layout): +1.6 MB of DMA (~4.5 us)
beats the ~20 us mesh-AllReduce latency floor for a tiny collective and leaves
the kernel with zero cross-core dependencies (immune to core start skew).

PE packing: the 9 conv taps are paired along kh so the contraction uses all
128 PE rows. x is DMAed twice into a [128, 784] tile; the quantized copy in
partitions 64:127 is written one padded row UP, so a single [128, 14, 28]
moving AP feeds tap (0,kw) from the top half and tap (1,kw) from the bottom
half of the same window. 3 pair-groups (K=128) + 3 kh=2 singles (K=64), each
split into two 392-column PSUM banks -> 12 matmuls, 4704 streamed columns
(vs 18 / 7056 unpaired).

Per-core pipeline:
  1. DMA xall [128,3136] (8 chunks, absmax partial-reduced as chunks land),
     x twice into [128,784], wpair [128,384], bias, consts.
  2. absmax -> PE-transpose -> row scalars; EMA thresholds; qscale = 127*(1/T)
     (bit-exact DVE reciprocal), s = T*(1/127), ss = s_x*s_w; broadcast via
     K=1 matmul with a ones row.
  3. Quantize (ACT Copy(v*qs+MAGIC) -> DVE (t-MAGIC) max -128 -> min 127 to
     bf16; MAGIC = 1.5*2^23 gives IEEE round-to-nearest-even = jnp.round).
  4. 6 matmul groups x 2 PSUM halves, accumulate.
  5. out = psum*ss + bias (DVE), DMA out.
"""

import os
import sys

import numpy as np

for _p in ("/opt/trn_rl_repo", "/root/.axon_site", "/root/.axon_site/_ro/trn_rl_repo",
           "/root/.axon_site/_ro/pypackages"):
    if os.path.isdir(_p) and _p not in sys.path:
        sys.path.append(_p)

from concourse import bacc, bass, masks, mybir, tile  # noqa: E402
from concourse.bass_utils import run_bass_kernel_spmd  # noqa: E402

F32 = mybir.dt.float32
BF16 = mybir.dt.bfloat16
AX = mybir.AxisListType
OP = mybir.AluOpType
ACTF = mybir.ActivationFunctionType

N_CORES = 8
CIN = 64
COUT = 64
K = 3
H = W = 28
P = H * W            # 784 pixels
PH = P // 2          # 392 per PSUM bank (14 output rows)
PAD = 30             # padded spatial edge
XD_F = PAD * PAD + PAD + 3 + 31  # x input row: padded image + pad + bias/consts
NCHUNK = 8           # xall DMA/reduce chunks
XALL_P = 128
XALL_F = (N_CORES * CIN * P) // XALL_P  # 3136
NG = 6               # matmul groups: 3 kh-pairs + 3 kh=2 singles
MAGIC = 12582912.0   # 1.5 * 2**23: fp32 add/sub round-to-nearest-even trick

EMA_MUL = 0.05
T_CONSTS = (2.85, 0.285)  # 0.95*T_FEATURE, 0.95*T_WEIGHT as fp32
INV127 = float(np.float32(1.0) / np.float32(127.0))


def _build():
    nc = bacc.Bacc(
        "TRN2",
        target_bir_lowering=False,
        debug=False,
        enable_asserts=True,
        num_devices=N_CORES,
    )
    xall_d = nc.dram_tensor("xall", [XALL_P, XALL_F], F32, kind="ExternalInput")
    x_d = nc.dram_tensor("x", [CIN, XD_F], F32, kind="ExternalInput")
    w_d = nc.dram_tensor("w", [2 * CIN, NG * COUT], F32, kind="ExternalInput")
    out_d = nc.dram_tensor("out", [COUT, P], F32, kind="ExternalOutput")

    with tile.TileContext(nc) as tc:
        with (
            tc.tile_pool(name="sbuf", bufs=1) as pool,
            tc.tile_pool(name="psum", bufs=1, space="PSUM") as psum,
        ):
            # ---- loads. Each dma_start spans all 128 partitions so its
            # descriptors fan out across all 16 SDMA engines (8 partitions
            # per engine). dma_start issue (DIRECT2D) costs ~0.6 us serial
            # per HWDGE sequencer, so spread issues across sync + scalar.
            # xall in 4 column-quarters so the absmax partial-reduce of
            # quarter q starts as soon as it lands.
            xall = pool.tile([XALL_P, XALL_F], F32)
            cf = XALL_F // 4
            for h in range(4):
                nc.sync.dma_start(out=xall[:, h * cf:(h + 1) * cf],
                                  in_=xall_d[:, h * cf:(h + 1) * cf])
            # x arrives host-padded [64, 964]: [30,30] pad-1 image, 30 zeros,
            # then bias in col 930 and (2.85, 0.285) in cols 931:933 of row 0.
            # Bottom half reads at +30 (one padded row up) so kh-pair matmul
            # windows hit tap kh+1 in partitions 64:128.
            x_sb = pool.tile([2 * CIN, XD_F - PAD], F32)
            nc.scalar.dma_start(out=x_sb[0:CIN, :], in_=x_d[:, 0:XD_F - PAD])
            nc.scalar.dma_start(out=x_sb[CIN:2 * CIN, :], in_=x_d[:, PAD:XD_F])
            w_sb = pool.tile([2 * CIN, NG * COUT], F32)
            nc.scalar.dma_start(out=w_sb[:], in_=w_d[:])
            bias_sb = x_sb[0:COUT, PAD * PAD + PAD:PAD * PAD + PAD + 1]
            crow = x_sb[0:1, PAD * PAD + PAD + 1:PAD * PAD + PAD + 3]

            ident = pool.tile([XALL_P, XALL_P], F32)
            masks.make_identity(nc, ident[:])

            # ---- absmax: w first (small DMA, runs during xall transfer),
            # then one partial per xall column-quarter (overlaps its DMA)
            pack = pool.tile([XALL_P, 2], F32)
            nc.vector.tensor_reduce(out=pack[:, 1:2], in_=w_sb[:], axis=AX.X,
                                    op=OP.max, apply_absolute_value=True)
            parts = pool.tile([XALL_P, 4], F32)
            for h in range(4):
                nc.vector.tensor_reduce(out=parts[:, h:h + 1],
                                        in_=xall[:, h * cf:(h + 1) * cf],
                                        axis=AX.X, op=OP.max,
                                        apply_absolute_value=True)
            nc.vector.tensor_reduce(out=pack[:, 0:1], in_=parts[:], axis=AX.X,
                                    op=OP.max)

            rowT = psum.tile([2, XALL_P], F32)
            nc.tensor.transpose(rowT[:], pack[:], ident[:])
            colmax = pool.tile([2, 1], F32)
            nc.vector.tensor_reduce(out=colmax[:], in_=rowT[:], axis=AX.X,
                                    op=OP.max)
            lrow_ps = psum.tile([1, 2], F32)
            nc.tensor.transpose(lrow_ps[:], colmax[:], ident[0:2, 0:2])
            grow = pool.tile([1, 2], F32)
            nc.vector.tensor_copy(grow[:], lrow_ps[:])

            # ---- scalar math in row form (partition 0)
            # T = grow*0.05 + (2.85, 0.285); two ops to force fp32 rounding
            t1 = pool.tile([1, 2], F32)
            nc.vector.tensor_scalar(out=t1[:], in0=grow[:], scalar1=EMA_MUL,
                                    scalar2=None, op0=OP.mult)
            trow = pool.tile([1, 2], F32)
            nc.vector.tensor_tensor(out=trow[:], in0=t1[:], in1=crow,
                                    op=OP.add)
            # qscale = 127*(1/T); s = T*(1/127); ss = s_x*s_w
            recip = pool.tile([1, 2], F32)
            nc.vector.reciprocal(recip[:], trow[:])
            brow = pool.tile([1, 3], F32)
            nc.vector.tensor_scalar(out=brow[:, 0:2], in0=recip[:],
                                    scalar1=127.0, scalar2=None, op0=OP.mult)
            srow = pool.tile([1, 2], F32)
            nc.vector.tensor_scalar(out=srow[:], in0=trow[:], scalar1=INV127,
                                    scalar2=None, op0=OP.mult)
            nc.vector.tensor_tensor(out=brow[:, 2:3], in0=srow[:, 0:1],
                                    in1=srow[:, 1:2], op=OP.mult)

            # ---- broadcast (qs_x, qs_w, ss) to all 128 partitions, K=1 matmul
            ones = pool.tile([1, XALL_P], F32)
            nc.vector.memset(ones[:], 1.0)
            scl_ps = psum.tile([XALL_P, 3], F32)
            nc.tensor.matmul(scl_ps[:], ones[:], brow[:])
            scales = pool.tile([XALL_P, 3], F32)
            nc.vector.tensor_copy(scales[:], scl_ps[:])

            # ---- quantize x: fully contiguous ops (padding quantizes to 0)
            tx = pool.tile([2 * CIN, PAD * PAD], F32)
            nc.scalar.activation(tx[:], x_sb[:, 0:PAD * PAD], ACTF.Copy,
                                 bias=MAGIC, scale=scales[:, 0:1])
            rx = pool.tile([2 * CIN, PAD * PAD], F32)
            nc.vector.tensor_scalar(out=rx[:], in0=tx[:], scalar1=MAGIC,
                                    scalar2=-128.0, op0=OP.subtract, op1=OP.max)
            qx2f = pool.tile([2 * CIN, PAD * PAD], BF16)
            nc.vector.tensor_scalar(out=qx2f[:], in0=rx[:], scalar1=127.0,
                                    scalar2=None, op0=OP.min)
            qx2 = qx2f[:].rearrange("p (h w) -> p h w", h=PAD)

            # ---- quantize w (ACT + DVE; gpsimd elementwise would contend
            # with DVE for SBUF ports and slow both)
            tw = pool.tile([2 * CIN, NG * COUT], F32)
            nc.scalar.activation(tw[:], w_sb[:], ACTF.Copy,
                                 bias=MAGIC, scale=scales[:, 1:2])
            rw = pool.tile([2 * CIN, NG * COUT], F32)
            nc.vector.tensor_scalar(out=rw[:], in0=tw[:], scalar1=MAGIC,
                                    scalar2=-128.0, op0=OP.subtract, op1=OP.max)
            qw = pool.tile([2 * CIN, NG * COUT], BF16)
            nc.vector.tensor_scalar(out=qw[:], in0=rw[:], scalar1=127.0,
                                    scalar2=None, op0=OP.min)

            # ---- conv: 3 kh-pair groups (K=128) + 3 kh=2 singles (K=64)
            ph0 = psum.tile([COUT, PH], F32)
            ph1 = psum.tile([COUT, PH], F32)
            for g in range(NG):
                if g < 3:  # taps (0,kw) + (1,kw), kw = g
                    kh, kw, kp = 0, g, 2 * CIN
                else:      # tap (2,kw), kw = g - 3
                    kh, kw, kp = 2, g - 3, CIN
                lhsT = qw[0:kp, g * COUT:(g + 1) * COUT]
                nc.tensor.matmul(
                    ph0[:], lhsT, qx2[0:kp, kh:kh + 14, kw:kw + W],
                    start=(g == 0), stop=(g == NG - 1))
                nc.tensor.matmul(
                    ph1[:], lhsT, qx2[0:kp, kh + 14:kh + 28, kw:kw + W],
                    start=(g == 0), stop=(g == NG - 1))

            # ---- epilogue: out = psum*ss + bias
            out_sb = pool.tile([COUT, P], F32)
            nc.vector.tensor_scalar(out=out_sb[:, 0:PH], in0=ph0[:],
                                    scalar1=scales[0:COUT, 2:3],
                                    scalar2=bias_sb,
                                    op0=OP.mult, op1=OP.add)
            nc.vector.tensor_scalar(out=out_sb[:, PH:P], in0=ph1[:],
                                    scalar1=scales[0:COUT, 2:3],
                                    scalar2=bias_sb,
                                    op0=OP.mult, op1=OP.add)
            nc.sync.dma_start(out=out_d[:], in_=out_sb[:])

    nc.compile()
    return nc


_NC = None


def _get_nc():
    global _NC
    if _NC is None:
        _NC = _build()
    return _NC


def _prep_in_maps(x, weight, bias):
    x = np.ascontiguousarray(x, dtype=np.float32)
    xall = x.reshape(XALL_P, XALL_F)
    # host-padded per-image layout [64, XD_F]: pad-1 image + 30 zeros, then
    # bias in col 930 and (2.85, 0.285) in cols 931:933 of row 0
    xpad = np.zeros((N_CORES, CIN, XD_F), dtype=np.float32)
    xpad[:, :, :PAD * PAD].reshape(N_CORES, CIN, PAD, PAD)[
        :, :, 1:1 + H, 1:1 + W] = x.reshape(N_CORES, CIN, H, W)
    xpad[:, :, PAD * PAD + PAD] = np.asarray(bias, dtype=np.float32)[None, :]
    xpad[:, 0, PAD * PAD + PAD + 1] = T_CONSTS[0]
    xpad[:, 0, PAD * PAD + PAD + 2] = T_CONSTS[1]
    # wpair [2*Cin, 6*Cout]: groups 0-2 pair taps (0,kw)/(1,kw) across the
    # partition halves; groups 3-5 hold tap (2,kw) in the top half only.
    wpair = np.zeros((2 * CIN, NG * COUT), dtype=np.float32)
    wt = np.transpose(weight, (1, 2, 3, 0))  # [Cin, kh, kw, Cout]
    for g in range(3):
        wpair[0:CIN, g * COUT:(g + 1) * COUT] = wt[:, 0, g, :]
        wpair[CIN:2 * CIN, g * COUT:(g + 1) * COUT] = wt[:, 1, g, :]
        wpair[0:CIN, (3 + g) * COUT:(4 + g) * COUT] = wt[:, 2, g, :]
    in_maps = []
    for b in range(N_CORES):
        in_maps.append({
            "xall": xall,
            "x": xpad[b],
            "w": wpair,
        })
    return in_maps


def _check_lut(lut):
    idx = np.arange(-128, 128, dtype=np.float32)
    expect = np.outer(idx, idx)
    if not np.array_equal(np.asarray(lut, dtype=np.float32), expect):
        raise ValueError(
            "lut is not the exact int8 product table; this kernel's PE-matmul "
            "formulation only applies to the exact-product LUT.")


def kernel(x, weight, bias, lut):
    _check_lut(lut)
    nc = _get_nc()
    in_maps = _prep_in_maps(np.asarray(x), np.asarray(weight), np.asarray(bias))
    res = run_bass_kernel_spmd(nc, in_maps, core_ids=list(range(N_CORES)))
    out = np.empty((N_CORES, COUT, H, W), dtype=np.float32)
    for b in range(N_CORES):
        out[b] = res.results[b]["out"].reshape(COUT, H, W)
    return out
